# revision 18
# baseline (speedup 1.0000x reference)
"""Causal dot-product attention (B=4, H=16, S=2048, D=128) on 8 TRN2 NeuronCores.

Sharding: batch*heads = 64 (b,h) pairs -> 8 heads per core (head parallel, no
communication). v2 design, evolved from the 193us baseline after trace analysis
showed the Scalar (ACT) engine's exp() was the pacer (161us of ACTIVATE) with
PE at 160us and neither fully busy:

  - Q,K,V are pre-laid-out on host in bf16 (Q,K transposed to [D=128, S] so
    the contraction dim is on partitions; V packed [kpos=128, kblock, D+1]
    with a ones column so PV's matmul produces the softmax denominator free).
    bf16 QK adds ~0.2% score noise - negligible vs the 2e-2 gate - and halves
    load DMA.
  - exp() is split across THREE engines: ~2/3 of 1024-col score chunks use the
    exact ACT spline exp; the rest run a one-instruction Schraudolph exp2 on
    the Vector (DVE) and GpSimd (Pool) engines: int16(st*A + B) bit-viewed as
    bf16 IS exp(scale*st) to ~2% (variance-optimal bias; softmax cancels the
    mean error exactly; measured ~1% end-to-end at this mix).
  - PV unchanged: p-tile stationary (bf16 -> fast weight load), rhs = V_aug
    [128,129], PSUM-accumulated per 256-q-col tile, deferred 2 chunks so the
    in-order PE queue never head-blocks on an in-flight exp.
  - Normalize: one reciprocal [128,2] + one scalar_tensor_tensor with a
    stride-0 broadcast of 1/l per q-block, bf16 output, single DMA per q-tile.
  - Output returned bf16, cast to fp32 on host (adds ~0.2% rounding).
"""

import math
import sys
from contextlib import ExitStack

import numpy as np

for _p in ("/opt/trn_rl_repo", "/root/.axon_site/_ro/trn_rl_repo"):
    if _p not in sys.path:
        sys.path.append(_p)

import ml_dtypes

import concourse.bass as bass
import concourse.tile as tile
from concourse import bacc, mybir
from concourse.bass_utils import run_bass_kernel_spmd

F32 = mybir.dt.float32
BF16 = mybir.dt.bfloat16
I16 = mybir.dt.int16
AF = mybir.ActivationFunctionType
Alu = mybir.AluOpType

# Problem constants (hardcoded; kernel.py must be self-contained).
B, H, S, D = 4, 16, 2048, 128
P = 128
N_CORES = 8
NH = (B * H) // N_CORES  # heads per core = 8
NKB = S // P  # 16 k-blocks per head
SCALE = 1.0 / math.sqrt(128.0)  # D_MODEL = 128

QTW = 256  # q-tile width
ST_COLS = 1024  # score-chunk width = one exp instruction (2 PSUM banks)
LAG = 2  # chunks of PV deferral

# Schraudolph exp2-in-bf16 constants: int16(st*SCH_A + SCH_B) bitcast to bf16
# approximates exp(SCALE*st). Bias tuned numerically for minimum error
# VARIANCE (softmax cancels the mean): delta = -2.5 over the 127*128 nominal,
# +0.5 to center truncation.
SCH_A = SCALE * (1.0 / math.log(2.0)) * 128.0
SCH_B = 127.0 * 128.0 + 0.5 - 2.5

# Per-head engine assignment for the 18 exp chunks: A=ACT exact spline,
# D=DVE Schraudolph. 12/6 split -> 33% approx mass. (Pool/GPSIMD cannot read
# PSUM, so it gets half the mask multiplies + output stores instead of exp.)
ENG_PATTERN = "AADAADAADAADAADDAA"


def build_nc(nh=NH, s=S):
    nqt = s // QTW  # q-tiles per head = 8
    n_chunks = ((s // P) * ((s // P) + 2) // 2 * P) // ST_COLS  # 18/head

    nc = bacc.Bacc("TRN2", target_bir_lowering=False, debug=False,
                   enable_asserts=False)
    qt_d = nc.declare_dram_parameter("qt", [nh, P, s], BF16, isOutput=False).ap()
    kt_d = nc.declare_dram_parameter("kt", [nh, P, s], BF16, isOutput=False).ap()
    v_d = nc.declare_dram_parameter("v", [nh, P, NKB * (D + 1)], BF16,
                                    isOutput=False).ap()
    mask_d = nc.declare_dram_parameter("mask", [P, P], BF16, isOutput=False).ap()
    out_d = nc.declare_dram_parameter("out", [nh, s, D], BF16, isOutput=True).ap()

    with tile.TileContext(nc) as tc, ExitStack() as ctx:
        kt_pool = ctx.enter_context(tc.tile_pool(name="kt_pool", bufs=2))
        qt_pool = ctx.enter_context(tc.tile_pool(name="qt_pool", bufs=2))
        v_pool = ctx.enter_context(tc.tile_pool(name="v_pool", bufs=2))
        pt_pool = ctx.enter_context(tc.tile_pool(name="pt_pool", bufs=6))
        st_pool = ctx.enter_context(tc.tile_pool(name="st_pool", bufs=3,
                                                 space="PSUM"))
        acc_pool = ctx.enter_context(tc.tile_pool(name="acc_pool", bufs=2,
                                                  space="PSUM"))
        o_pool = ctx.enter_context(tc.tile_pool(name="o_pool", bufs=4))
        r_pool = ctx.enter_context(tc.tile_pool(name="r_pool", bufs=4))
        misc = ctx.enter_context(tc.tile_pool(name="misc", bufs=1))

        mask_t = misc.tile([P, P], BF16)

        # Streaming exp state. st fills with QK chunks; one exp instruction
        # (on the chunk's assigned engine) drains it to a bf16 pt tile.
        # Diagonal masks are emitted RIGHT AFTER the exp (so they're long done
        # when PV needs them); PV matmuls go through `pvq` and are woven
        # between QK matmuls, 2 per QK, LAG chunks later, so LDWEIGHTS always
        # hides under a running matmul and PV never head-blocks on exp.
        state = {"st": None, "fill": 0, "entries": [],
                 "pending": [], "chunk": 0, "mask_rr": 0, "store_rr": 0,
                 "pvq": []}

        def normalize(h, i, acc_t):
            r_t = r_pool.tile([P, 2], F32, tag="r", name="r_t")
            nc.vector.reciprocal(r_t[:], acc_t[:, 128:258:129])
            o_t = o_pool.tile([P, 2 * P], BF16, tag="o", name="o_t")
            in0 = acc_t[:].rearrange("p (b c) -> p b c", b=2)[:, :, 0:128]
            in1 = r_t[:].unsqueeze(2).broadcast_to([P, 2, P])
            nc.vector.scalar_tensor_tensor(
                o_t[:].rearrange("p (b c) -> p b c", b=2), in0, 1.0, in1,
                op0=Alu.mult, op1=Alu.mult)
            dst = out_d[h, i * QTW:(i + 1) * QTW, :].rearrange(
                "(b q) d -> q b d", b=2)
            q = nc.sync if state["store_rr"] % 2 else nc.gpsimd
            state["store_rr"] += 1
            q.dma_start(out=dst, in_=o_t[:].rearrange("p (b c) -> p b c", b=2))

        def expand_pv(group):
            """Turn a drained chunk into PV micro-ops on the pvq queue."""
            pt_bf, entries = group
            for (pos, g, j, acc_rec, v_t) in entries:
                ps = pt_bf[:, pos:pos + P]
                state["pvq"].append((ps, g, j, acc_rec, v_t))

        def drain_pv(n):
            pvq = state["pvq"]
            for _ in range(min(n, len(pvq))):
                ps, g, j, acc_rec, v_t = pvq.pop(0)
                acc_t, eh, i = acc_rec["acc"], acc_rec["h"], acc_rec["i"]
                sI = g - 2 * i
                start = not acc_rec["started"]
                acc_rec["started"] = True
                acc_rec["left"] -= 1
                stop = acc_rec["left"] == 0
                nc.tensor.matmul(acc_t[:, sI * 129:(sI + 1) * 129],
                                 lhsT=ps, rhs=v_t[:, j * 129:(j + 1) * 129],
                                 start=start, stop=stop)
                if stop:
                    normalize(eh, i, acc_t)

        def flush(final=False):
            pend = state["pending"]
            if state["fill"]:
                w = state["fill"]
                st_t = state["st"]
                eng = ENG_PATTERN[state["chunk"] % len(ENG_PATTERN)]
                state["chunk"] += 1
                if final:
                    # Tail trim: split the very last exp across ACT and DVE so
                    # the closing PV/normalize chain starts ~0.5us sooner.
                    pt_t = pt_pool.tile([P, ST_COLS], BF16, tag="pt", name="pt_t")
                    h1 = (w // 2) // P * P or P
                    nc.scalar.activation(pt_t[:, :h1], st_t[:, :h1], AF.Exp,
                                         bias=0.0, scale=SCALE)
                    if w > h1:
                        nc.vector.tensor_scalar(
                            pt_t[:, h1:w].bitcast(I16), st_t[:, h1:w],
                            SCH_A, SCH_B, Alu.mult, Alu.add)
                    pt_bf = pt_t[:]
                elif eng == "A":
                    pt_t = pt_pool.tile([P, ST_COLS], BF16, tag="pt", name="pt_t")
                    nc.scalar.activation(pt_t[:, :w], st_t[:, :w], AF.Exp,
                                         bias=0.0, scale=SCALE)
                    pt_bf = pt_t[:]
                else:
                    pt_t = pt_pool.tile([P, ST_COLS], I16, tag="pt", name="pt_t")
                    nc.vector.tensor_scalar(pt_t[:, :w], st_t[:, :w],
                                            SCH_A, SCH_B, Alu.mult, Alu.add)
                    pt_bf = pt_t[:].bitcast(BF16)
                # Masks now, on alternating Vector/Pool, so they never gate PV.
                for (pos, g, j, acc_rec, v_t) in state["entries"]:
                    if g == j:
                        ps = pt_bf[:, pos:pos + P]
                        eng_m = (nc.vector if state["mask_rr"] % 2
                                 else nc.gpsimd)
                        state["mask_rr"] += 1
                        eng_m.tensor_mul(ps, ps, mask_t[:])
                pend.append((pt_bf, state["entries"]))
            lag = 0 if final else LAG
            while len(pend) > lag:
                expand_pv(pend.pop(0))
            if final:
                drain_pv(len(state["pvq"]))
            state.update(st=None, fill=0, entries=[], pending=pend)

        def emit_qk(h, i, j, q0, width, kt_t, qt_t, acc_rec, v_t):
            """One QK piece: q-cols [q0, q0+width) against k-block j, split at
            PSUM bank (512-col) and chunk boundaries."""
            done = 0
            while done < width:
                if state["fill"] == 0:
                    state["st"] = st_pool.tile([P, ST_COLS], F32,
                                               tag="st", name="st_t")
                pos = state["fill"]
                w = min(width - done, 512 - pos % 512, ST_COLS - pos)
                nc.tensor.matmul(state["st"][:, pos:pos + w],
                                 lhsT=kt_t[:, j * P:(j + 1) * P],
                                 rhs=qt_t[:, q0 + done:q0 + done + w],
                                 start=True, stop=True)
                drain_pv(2)
                for b in range(w // P):
                    g = (q0 + done) // P + b
                    state["entries"].append((pos + b * P, g, j, acc_rec, v_t))
                state["fill"] += w
                done += w
                if state["fill"] == ST_COLS:
                    flush()

        stash = {}

        def start_head(h):
            if h in stash:
                return
            kt_t = kt_pool.tile([P, s], BF16, tag="kt", name="kt_t")
            qt_t = qt_pool.tile([P, s], BF16, tag="qt", name="qt_t")
            v_t = v_pool.tile([P, NKB * (D + 1)], BF16, tag="v", name="v_t")
            stash[h] = (kt_t, qt_t, v_t)
            if h == 0:
                # Fast start: just enough for the first chunk, then the mask
                # and V, then kt/qt interleaved at the 2:1 rate the causal
                # triangle consumes them (q-tile i needs kt[0:(2i+2)*128] but
                # only qt[i*256:(i+1)*256]).
                nc.sync.dma_start(out=qt_t[:, :QTW], in_=qt_d[0, :, :QTW])
                nc.sync.dma_start(out=kt_t[:, :QTW], in_=kt_d[0, :, :QTW])
                nc.sync.dma_start(out=mask_t[:], in_=mask_d)
                nc.sync.dma_start(out=v_t[:, :2 * 129], in_=v_d[0, :, :2 * 129])
                nc.sync.dma_start(out=kt_t[:, QTW:2 * QTW],
                                  in_=kt_d[0, :, QTW:2 * QTW])
                nc.sync.dma_start(out=qt_t[:, QTW:2 * QTW],
                                  in_=qt_d[0, :, QTW:2 * QTW])
                nc.sync.dma_start(out=kt_t[:, 2 * QTW:1024],
                                  in_=kt_d[0, :, 2 * QTW:1024])
                nc.sync.dma_start(out=qt_t[:, 2 * QTW:1024],
                                  in_=qt_d[0, :, 2 * QTW:1024])
                nc.sync.dma_start(out=v_t[:, 2 * 129:], in_=v_d[0, :, 2 * 129:])
                nc.sync.dma_start(out=kt_t[:, 1024:], in_=kt_d[0, :, 1024:])
                nc.sync.dma_start(out=qt_t[:, 1024:], in_=qt_d[0, :, 1024:])
            else:
                for c in range(0, s, 1024):
                    nc.sync.dma_start(out=kt_t[:, c:c + 1024],
                                      in_=kt_d[h, :, c:c + 1024])
                nc.sync.dma_start(out=v_t[:], in_=v_d[h])
                for c in range(0, s, 1024):
                    nc.sync.dma_start(out=qt_t[:, c:c + 1024],
                                      in_=qt_d[h, :, c:c + 1024])
            return

        for h in range(nh):
            start_head(h)
            kt_t, qt_t, v_t = stash[h]
            for i in range(nqt):
                if i == 3 and h + 1 < nh:
                    start_head(h + 1)
                acc_t = acc_pool.tile([P, 2 * 129], F32, tag="acc",
                                      name="acc_t")
                acc_rec = {"acc": acc_t, "h": h, "i": i, "started": False,
                           "left": 4 * i + 3}
                for j in range(2 * i + 1):  # full 256-wide causal k-blocks
                    emit_qk(h, i, j, i * QTW, QTW, kt_t, qt_t, acc_rec, v_t)
                # j = 2i+1: only the upper q-half survives the causal mask.
                # Emit the 128 live columns, then pad the chunk by 128 unread
                # garbage columns so later pieces stay 256-aligned (a matmul
                # dst must not straddle a PSUM bank). exp over the pad is
                # wasted but the QK matmul columns are saved.
                emit_qk(h, i, 2 * i + 1, i * QTW + P, P, kt_t, qt_t, acc_rec,
                        v_t)
                state["fill"] += P
                if state["fill"] == ST_COLS:
                    flush()
        flush(final=True)
    nc.compile()
    return nc


_NC = None


def _get_nc():
    global _NC
    if _NC is None:
        _NC = build_nc()
    return _NC


def prepare_in_maps(Q, K, V):
    """Shard + lay out full [B,H,S,D] inputs into per-core in_maps."""
    Qf = np.asarray(Q, dtype=np.float32).reshape(B * H, S, D)
    Kf = np.asarray(K, dtype=np.float32).reshape(B * H, S, D)
    Vf = np.asarray(V, dtype=np.float32).reshape(B * H, S, D)
    mask = np.triu(np.ones((P, P), dtype=np.float32)).astype(ml_dtypes.bfloat16)
    in_maps = []
    for c in range(N_CORES):
        hs = slice(c * NH, (c + 1) * NH)
        qt = np.ascontiguousarray(
            Qf[hs].transpose(0, 2, 1)).astype(ml_dtypes.bfloat16)  # [NH, D, S]
        kt = np.ascontiguousarray(
            Kf[hs].transpose(0, 2, 1)).astype(ml_dtypes.bfloat16)  # [NH, D, S]
        # V: [NH, S, D] -> [NH, kblock, kpos, D] -> [NH, kpos, kblock, D+1]
        vv = Vf[hs].reshape(NH, NKB, P, D).transpose(0, 2, 1, 3)
        v_aug = np.ones((NH, P, NKB, D + 1), dtype=ml_dtypes.bfloat16)
        v_aug[..., :D] = vv.astype(ml_dtypes.bfloat16)
        in_maps.append({"qt": qt, "kt": kt,
                        "v": v_aug.reshape(NH, P, NKB * (D + 1)), "mask": mask})
    return in_maps


def gather_out(results):
    out = np.concatenate([np.asarray(r["out"]).astype(np.float32)
                          for r in results], axis=0)  # [64, S, D]
    return out.reshape(B, H, S, D)


def kernel(Q, K, V):
    in_maps = prepare_in_maps(Q, K, V)
    nc = _get_nc()
    res = run_bass_kernel_spmd(nc, in_maps, core_ids=list(range(N_CORES)))
    return gather_out(res.results)


# revision 22
# speedup vs baseline: 1.0171x; 1.0171x over previous
"""Causal dot-product attention (B=4, H=16, S=2048, D=128) on 8 TRN2 NeuronCores.

Sharding: batch*heads = 64 (b,h) pairs -> 8 heads per core (head parallel, no
communication). v2 design, evolved from the 193us baseline after trace analysis
showed the Scalar (ACT) engine's exp() was the pacer (161us of ACTIVATE) with
PE at 160us and neither fully busy:

  - Q,K,V are pre-laid-out on host in bf16 (Q,K transposed to [D=128, S] so
    the contraction dim is on partitions; V packed [kpos=128, kblock, D+1]
    with a ones column so PV's matmul produces the softmax denominator free).
    bf16 QK adds ~0.2% score noise - negligible vs the 2e-2 gate - and halves
    load DMA.
  - exp() is split across THREE engines: ~2/3 of 1024-col score chunks use the
    exact ACT spline exp; the rest run a one-instruction Schraudolph exp2 on
    the Vector (DVE) and GpSimd (Pool) engines: int16(st*A + B) bit-viewed as
    bf16 IS exp(scale*st) to ~2% (variance-optimal bias; softmax cancels the
    mean error exactly; measured ~1% end-to-end at this mix).
  - PV unchanged: p-tile stationary (bf16 -> fast weight load), rhs = V_aug
    [128,129], PSUM-accumulated per 256-q-col tile, deferred 2 chunks so the
    in-order PE queue never head-blocks on an in-flight exp.
  - Normalize: one reciprocal [128,2] + one scalar_tensor_tensor with a
    stride-0 broadcast of 1/l per q-block, bf16 output, single DMA per q-tile.
  - Output returned bf16, cast to fp32 on host (adds ~0.2% rounding).
"""

import math
import sys
from contextlib import ExitStack

import numpy as np

for _p in ("/opt/trn_rl_repo", "/root/.axon_site/_ro/trn_rl_repo"):
    if _p not in sys.path:
        sys.path.append(_p)

import ml_dtypes

import concourse.bass as bass
import concourse.tile as tile
from concourse import bacc, mybir
from concourse.bass_utils import run_bass_kernel_spmd

F32 = mybir.dt.float32
BF16 = mybir.dt.bfloat16
I16 = mybir.dt.int16
AF = mybir.ActivationFunctionType
Alu = mybir.AluOpType

# Problem constants (hardcoded; kernel.py must be self-contained).
B, H, S, D = 4, 16, 2048, 128
P = 128
N_CORES = 8
NH = (B * H) // N_CORES  # heads per core = 8
NKB = S // P  # 16 k-blocks per head
SCALE = 1.0 / math.sqrt(128.0)  # D_MODEL = 128

QTW = 256  # q-tile width
ST_COLS = 1024  # score-chunk width = one exp instruction (2 PSUM banks)
LAG = 2  # chunks of PV deferral

# Schraudolph exp2-in-bf16 constants: int16(st*SCH_A + SCH_B) bitcast to bf16
# approximates exp(SCALE*st). Bias tuned numerically for minimum error
# VARIANCE (softmax cancels the mean): delta = -2.5 over the 127*128 nominal,
# +0.5 to center truncation.
SCH_A = SCALE * (1.0 / math.log(2.0)) * 128.0
SCH_B = 127.0 * 128.0 + 0.5 - 2.5

# Per-head engine assignment for the 18 exp chunks: A=ACT exact spline,
# D=DVE Schraudolph. 12/6 split -> 33% approx mass. (Pool/GPSIMD cannot read
# PSUM, so it gets half the mask multiplies + output stores instead of exp.)
ENG_PATTERN = "AADAADAADAADAADAAD"


def build_nc(nh=NH, s=S):
    nqt = s // QTW  # q-tiles per head = 8
    n_chunks = ((s // P) * ((s // P) + 2) // 2 * P) // ST_COLS  # 18/head

    nc = bacc.Bacc("TRN2", target_bir_lowering=False, debug=False,
                   enable_asserts=False)
    qt_d = nc.declare_dram_parameter("qt", [nh, P, s], BF16, isOutput=False).ap()
    kt_d = nc.declare_dram_parameter("kt", [nh, P, s], BF16, isOutput=False).ap()
    v_d = nc.declare_dram_parameter("v", [nh, P, NKB * (D + 1)], BF16,
                                    isOutput=False).ap()
    mask_d = nc.declare_dram_parameter("mask", [P, P], BF16, isOutput=False).ap()
    out_d = nc.declare_dram_parameter("out", [nh, s, D], BF16, isOutput=True).ap()

    with tile.TileContext(nc) as tc, ExitStack() as ctx:
        kt_pool = ctx.enter_context(tc.tile_pool(name="kt_pool", bufs=2))
        qt_pool = ctx.enter_context(tc.tile_pool(name="qt_pool", bufs=2))
        v_pool = ctx.enter_context(tc.tile_pool(name="v_pool", bufs=2))
        pt_pool = ctx.enter_context(tc.tile_pool(name="pt_pool", bufs=6))
        st_pool = ctx.enter_context(tc.tile_pool(name="st_pool", bufs=3,
                                                 space="PSUM"))
        acc_pool = ctx.enter_context(tc.tile_pool(name="acc_pool", bufs=2,
                                                  space="PSUM"))
        o_pool = ctx.enter_context(tc.tile_pool(name="o_pool", bufs=4))
        r_pool = ctx.enter_context(tc.tile_pool(name="r_pool", bufs=4))
        misc = ctx.enter_context(tc.tile_pool(name="misc", bufs=1))

        mask_t = misc.tile([P, P], BF16)

        # Streaming exp state. st fills with QK chunks; one exp instruction
        # (on the chunk's assigned engine) drains it to a bf16 pt tile.
        # Diagonal masks are emitted RIGHT AFTER the exp (so they're long done
        # when PV needs them); PV matmuls go through `pvq` and are woven
        # between QK matmuls, 2 per QK, LAG chunks later, so LDWEIGHTS always
        # hides under a running matmul and PV never head-blocks on exp.
        state = {"st": None, "fill": 0, "entries": [],
                 "pending": [], "chunk": 0, "mask_rr": 0, "store_rr": 0,
                 "pvq": []}

        def normalize(h, i, acc_t):
            r_t = r_pool.tile([P, 2], F32, tag="r", name="r_t")
            nc.vector.reciprocal(r_t[:], acc_t[:, 128:258:129])
            o_t = o_pool.tile([P, 2 * P], BF16, tag="o", name="o_t")
            in0 = acc_t[:].rearrange("p (b c) -> p b c", b=2)[:, :, 0:128]
            in1 = r_t[:].unsqueeze(2).broadcast_to([P, 2, P])
            nc.vector.scalar_tensor_tensor(
                o_t[:].rearrange("p (b c) -> p b c", b=2), in0, 1.0, in1,
                op0=Alu.mult, op1=Alu.mult)
            dst = out_d[h, i * QTW:(i + 1) * QTW, :].rearrange(
                "(b q) d -> q b d", b=2)
            q = nc.sync if state["store_rr"] % 2 else nc.gpsimd
            state["store_rr"] += 1
            q.dma_start(out=dst, in_=o_t[:].rearrange("p (b c) -> p b c", b=2))

        def expand_pv(group):
            """Turn a drained chunk into PV micro-ops on the pvq queue."""
            pt_bf, entries = group
            for (pos, g, j, acc_rec, v_t) in entries:
                ps = pt_bf[:, pos:pos + P]
                state["pvq"].append((ps, g, j, acc_rec, v_t))

        def drain_pv(n):
            pvq = state["pvq"]
            for _ in range(min(n, len(pvq))):
                ps, g, j, acc_rec, v_t = pvq.pop(0)
                acc_t, eh, i = acc_rec["acc"], acc_rec["h"], acc_rec["i"]
                sI = g - 2 * i
                start = not acc_rec["started"]
                acc_rec["started"] = True
                acc_rec["left"] -= 1
                stop = acc_rec["left"] == 0
                nc.tensor.matmul(acc_t[:, sI * 129:(sI + 1) * 129],
                                 lhsT=ps, rhs=v_t[:, j * 129:(j + 1) * 129],
                                 start=start, stop=stop)
                if stop:
                    normalize(eh, i, acc_t)

        def flush(final=False):
            pend = state["pending"]
            if state["fill"]:
                w = state["fill"]
                st_t = state["st"]
                eng = ENG_PATTERN[state["chunk"] % len(ENG_PATTERN)]
                state["chunk"] += 1
                if final:
                    # Tail trim: split the very last exp across ACT and DVE so
                    # the closing PV/normalize chain starts ~0.5us sooner.
                    pt_t = pt_pool.tile([P, ST_COLS], BF16, tag="pt", name="pt_t")
                    h1 = (w // 2) // P * P or P
                    nc.scalar.activation(pt_t[:, :h1], st_t[:, :h1], AF.Exp,
                                         bias=0.0, scale=SCALE)
                    if w > h1:
                        nc.vector.tensor_scalar(
                            pt_t[:, h1:w].bitcast(I16), st_t[:, h1:w],
                            SCH_A, SCH_B, Alu.mult, Alu.add)
                    pt_bf = pt_t[:]
                elif eng == "A":
                    pt_t = pt_pool.tile([P, ST_COLS], BF16, tag="pt", name="pt_t")
                    nc.scalar.activation(pt_t[:, :w], st_t[:, :w], AF.Exp,
                                         bias=0.0, scale=SCALE)
                    pt_bf = pt_t[:]
                else:
                    pt_t = pt_pool.tile([P, ST_COLS], I16, tag="pt", name="pt_t")
                    nc.vector.tensor_scalar(pt_t[:, :w], st_t[:, :w],
                                            SCH_A, SCH_B, Alu.mult, Alu.add)
                    pt_bf = pt_t[:].bitcast(BF16)
                # Masks now, on alternating Vector/Pool, so they never gate PV.
                for (pos, g, j, acc_rec, v_t) in state["entries"]:
                    if g == j:
                        ps = pt_bf[:, pos:pos + P]
                        eng_m = (nc.vector if state["mask_rr"] % 2
                                 else nc.gpsimd)
                        state["mask_rr"] += 1
                        eng_m.tensor_mul(ps, ps, mask_t[:])
                pend.append((pt_bf, state["entries"]))
            lag = 0 if final else LAG
            while len(pend) > lag:
                expand_pv(pend.pop(0))
            if final:
                drain_pv(len(state["pvq"]))
            state.update(st=None, fill=0, entries=[], pending=pend)

        def emit_qk(h, i, j, q0, width, kt_t, qt_t, acc_rec, v_t):
            """One QK piece: q-cols [q0, q0+width) against k-block j, split at
            PSUM bank (512-col) and chunk boundaries."""
            done = 0
            while done < width:
                if state["fill"] == 0:
                    state["st"] = st_pool.tile([P, ST_COLS], F32,
                                               tag="st", name="st_t")
                pos = state["fill"]
                w = min(width - done, 512 - pos % 512, ST_COLS - pos)
                nc.tensor.matmul(state["st"][:, pos:pos + w],
                                 lhsT=kt_t[:, j * P:(j + 1) * P],
                                 rhs=qt_t[:, q0 + done:q0 + done + w],
                                 start=True, stop=True)
                drain_pv(2)
                for b in range(w // P):
                    g = (q0 + done) // P + b
                    state["entries"].append((pos + b * P, g, j, acc_rec, v_t))
                state["fill"] += w
                done += w
                if state["fill"] == ST_COLS:
                    flush()

        stash = {}

        def start_head(h):
            if h in stash:
                return
            kt_t = kt_pool.tile([P, s], BF16, tag="kt", name="kt_t")
            qt_t = qt_pool.tile([P, s], BF16, tag="qt", name="qt_t")
            v_t = v_pool.tile([P, NKB * (D + 1)], BF16, tag="v", name="v_t")
            stash[h] = (kt_t, qt_t, v_t)
            if h == 0:
                # Fast start: 256-col pieces issued in the exact order the
                # causal triangle consumes them (chunk c needs qt up to
                # ~256*(c+2) but kt grows twice as fast), then the bulk.
                nc.sync.dma_start(out=qt_t[:, :QTW], in_=qt_d[0, :, :QTW])
                nc.sync.dma_start(out=kt_t[:, :QTW], in_=kt_d[0, :, :QTW])
                nc.sync.dma_start(out=qt_t[:, QTW:2 * QTW],
                                  in_=qt_d[0, :, QTW:2 * QTW])
                nc.sync.dma_start(out=mask_t[:], in_=mask_d)
                nc.sync.dma_start(out=v_t[:, :2 * 129], in_=v_d[0, :, :2 * 129])
                nc.sync.dma_start(out=kt_t[:, QTW:2 * QTW],
                                  in_=kt_d[0, :, QTW:2 * QTW])
                nc.sync.dma_start(out=qt_t[:, 2 * QTW:3 * QTW],
                                  in_=qt_d[0, :, 2 * QTW:3 * QTW])
                nc.sync.dma_start(out=kt_t[:, 2 * QTW:3 * QTW],
                                  in_=kt_d[0, :, 2 * QTW:3 * QTW])
                nc.sync.dma_start(out=qt_t[:, 3 * QTW:4 * QTW],
                                  in_=qt_d[0, :, 3 * QTW:4 * QTW])
                nc.sync.dma_start(out=kt_t[:, 3 * QTW:5 * QTW],
                                  in_=kt_d[0, :, 3 * QTW:5 * QTW])
                nc.sync.dma_start(out=qt_t[:, 4 * QTW:6 * QTW],
                                  in_=qt_d[0, :, 4 * QTW:6 * QTW])
                nc.sync.dma_start(out=kt_t[:, 5 * QTW:7 * QTW],
                                  in_=kt_d[0, :, 5 * QTW:7 * QTW])
                nc.sync.dma_start(out=qt_t[:, 6 * QTW:], in_=qt_d[0, :, 6 * QTW:])
                nc.sync.dma_start(out=v_t[:, 2 * 129:], in_=v_d[0, :, 2 * 129:])
                nc.sync.dma_start(out=kt_t[:, 7 * QTW:], in_=kt_d[0, :, 7 * QTW:])
            else:
                for c in range(0, s, 1024):
                    nc.sync.dma_start(out=kt_t[:, c:c + 1024],
                                      in_=kt_d[h, :, c:c + 1024])
                nc.sync.dma_start(out=v_t[:], in_=v_d[h])
                for c in range(0, s, 1024):
                    nc.sync.dma_start(out=qt_t[:, c:c + 1024],
                                      in_=qt_d[h, :, c:c + 1024])
            return

        for h in range(nh):
            start_head(h)
            kt_t, qt_t, v_t = stash[h]
            # Last head runs its q-tiles largest-first so the tail chain after
            # the final exp is the small q-tiles (few PV matmuls), not the
            # 16-matmul diagonal monster.
            order = range(nqt) if h + 1 < nh else reversed(range(nqt))
            for i in order:
                if i == 3 and h + 1 < nh:
                    start_head(h + 1)
                acc_t = acc_pool.tile([P, 2 * 129], F32, tag="acc",
                                      name="acc_t")
                acc_rec = {"acc": acc_t, "h": h, "i": i, "started": False,
                           "left": 4 * i + 3}
                for j in range(2 * i + 1):  # full 256-wide causal k-blocks
                    emit_qk(h, i, j, i * QTW, QTW, kt_t, qt_t, acc_rec, v_t)
                # j = 2i+1: only the upper q-half survives the causal mask.
                # Emit the 128 live columns, then pad the chunk by 128 unread
                # garbage columns so later pieces stay 256-aligned (a matmul
                # dst must not straddle a PSUM bank). exp over the pad is
                # wasted but the QK matmul columns are saved.
                emit_qk(h, i, 2 * i + 1, i * QTW + P, P, kt_t, qt_t, acc_rec,
                        v_t)
                state["fill"] += P
                if state["fill"] == ST_COLS:
                    flush()
        flush(final=True)
    nc.compile()
    return nc


_NC = None


def _get_nc():
    global _NC
    if _NC is None:
        _NC = build_nc()
    return _NC


def prepare_in_maps(Q, K, V):
    """Shard + lay out full [B,H,S,D] inputs into per-core in_maps."""
    Qf = np.asarray(Q, dtype=np.float32).reshape(B * H, S, D)
    Kf = np.asarray(K, dtype=np.float32).reshape(B * H, S, D)
    Vf = np.asarray(V, dtype=np.float32).reshape(B * H, S, D)
    mask = np.triu(np.ones((P, P), dtype=np.float32)).astype(ml_dtypes.bfloat16)
    in_maps = []
    for c in range(N_CORES):
        hs = slice(c * NH, (c + 1) * NH)
        qt = np.ascontiguousarray(
            Qf[hs].transpose(0, 2, 1)).astype(ml_dtypes.bfloat16)  # [NH, D, S]
        kt = np.ascontiguousarray(
            Kf[hs].transpose(0, 2, 1)).astype(ml_dtypes.bfloat16)  # [NH, D, S]
        # V: [NH, S, D] -> [NH, kblock, kpos, D] -> [NH, kpos, kblock, D+1]
        vv = Vf[hs].reshape(NH, NKB, P, D).transpose(0, 2, 1, 3)
        v_aug = np.ones((NH, P, NKB, D + 1), dtype=ml_dtypes.bfloat16)
        v_aug[..., :D] = vv.astype(ml_dtypes.bfloat16)
        in_maps.append({"qt": qt, "kt": kt,
                        "v": v_aug.reshape(NH, P, NKB * (D + 1)), "mask": mask})
    return in_maps


def gather_out(results):
    out = np.concatenate([np.asarray(r["out"]).astype(np.float32)
                          for r in results], axis=0)  # [64, S, D]
    return out.reshape(B, H, S, D)


def kernel(Q, K, V):
    in_maps = prepare_in_maps(Q, K, V)
    nc = _get_nc()
    res = run_bass_kernel_spmd(nc, in_maps, core_ids=list(range(N_CORES)))
    return gather_out(res.results)
